# revision 14
# baseline (speedup 1.0000x reference)
"""FARGAN vocoder Trainium2 Bass kernel.

Strategy: pure data parallelism — batch 64 sharded 8 rows/core across 8
NeuronCores. Each core runs: (1) the conv cond-net as big matmuls, (2) the
1600-step subframe recurrence with feature-major activations (weights
stationary on the PE), with the pitch-predictor gather done via indirect DMA
from the DRAM output buffer (which doubles as the excitation history).

Host precomputes (numpy/jax-cpu, exact f32 semantics): phase embeddings,
gains, gather offsets, cond-net input, and packs/transposes/pads weights.
"""
import sys, os, json

sys.path.insert(0, "/opt/trn_rl_repo")

import numpy as np

SUB, NSUB, FRAME, CND, FDIM, PEMB = 40, 4, 160, 256, 20, 64
FWC_IN = 240
B, T, NBF = 64, 404, 400
STEPS = NBF * NSUB            # 1600
NCORE, BPC = 8, 8             # cores, batch per core
CW = 3264                     # padded cond-net width (8*404 = 3232 + pad)

# ----------------------------------------------------------------------------
# BIR legalizer: this walrus build allows at most ONE sync wait per
# instruction; hoist extra waits onto same-engine NoOps placed just before.
# ----------------------------------------------------------------------------

def _legalize_bir_json(raw: bytes) -> bytes:
    d = json.loads(raw)
    ctr = [0]

    def mk_nop(engine, wait, debug):
        ctr[0] += 1
        return {"debug": debug, "engine": engine, "ins": [],
                "name": f"legal-nop-{ctr[0]}", "opcode": "NoOp", "outs": [],
                "sync_info": {"on_update": [], "on_wait": [wait]}}

    for f in d.get("functions", []):
        for b in f.get("blocks", []):
            out = []
            for inst in b["instructions"]:
                si = inst.get("sync_info")
                waits = (si or {}).get("on_wait") or []
                if len(waits) > 1:
                    for w in waits[:-1]:
                        out.append(mk_nop(inst["engine"], w, inst.get("debug", 0)))
                    si["on_wait"] = [waits[-1]]
                out.append(inst)
            b["instructions"] = out
    return json.dumps(d).encode()


# ----------------------------------------------------------------------------
# Host precompute
# ----------------------------------------------------------------------------

def _phase_embedding(period):
    """preal, pimag [B, STEPS*SUB] — computed with jax CPU, matching reference."""
    import jax, jax.numpy as jnp
    cpu = jax.devices("cpu")[0]
    with jax.default_device(cpu):
        periods = jnp.asarray(period[:, 3:-1]).astype(jnp.float32)
        nB = periods.shape[0]
        w0 = 2.0 * jnp.pi / periods
        shift = 2.0 * jnp.pi * jax.random.uniform(
            jax.random.key(42), (nB, 1), periods.dtype) / FRAME
        w0s = jnp.concatenate([shift, w0[:, :-1]], 1)
        cum = FRAME * jnp.cumsum(w0s, 1)
        fine = w0[:, :, None] * jnp.arange(FRAME, dtype=w0.dtype)
        emb = (cum[:, :, None] + fine).reshape(nB, -1)
        return np.asarray(jnp.cos(emb)), np.asarray(jnp.sin(emb))


def host_prep(inputs, steps=STEPS):
    """Returns list of per-core input maps."""
    f = {k: np.asarray(v) for k, v in inputs.items() if hasattr(v, "shape")}
    features, period = np.asarray(f["features"], np.float32), np.asarray(f["period"])
    nbf = steps // NSUB

    preal, pimag = _phase_embedding(period)
    pr = preal.reshape(B, STEPS, SUB)[:, :steps]
    pi = pimag.reshape(B, STEPS, SUB)[:, :steps]
    phs = np.concatenate([pr, pi], -1)                       # [B, steps, 80]

    gain = np.float32(0.03) * np.power(
        np.float32(10.0),
        np.float32(0.5) * features[:, 3:3 + nbf, 0] / np.float32(np.sqrt(18.0)))
    gain_s = np.repeat(gain, NSUB, axis=1).astype(np.float32)        # [B, steps]
    ginv_s = (np.float32(1.0) / (np.float32(1e-5) + gain_s)).astype(np.float32)

    pit = np.clip(period[:, 3:3 + nbf], SUB + 2, 254)
    o = (254 - pit).astype(np.int64)
    s_idx = np.arange(steps)
    opos = 40 * s_idx[None, :] + np.repeat(o, NSUB, axis=1)          # [B, steps]

    p_emb = f["pembed"][period]                                      # [B, T, 64]
    xc = np.concatenate([features, p_emb], -1).astype(np.float32)    # [B, T, 84]

    # ---------------- weights (shared across cores) ----------------
    W = {}
    W["wfd1"] = np.ascontiguousarray(f["w_fd1"].T).astype(np.float32)

    def tile_k(wT, ktiles):
        K, M = wT.shape
        out = np.zeros((ktiles, 128, M), np.float32)
        for t in range(ktiles):
            blk = wT[128 * t:128 * (t + 1)]
            out[t, :blk.shape[0]] = blk
        return out

    for name, k in (("k1", f["k_fc1"]), ("k2", f["k_fc2"])):
        arr = np.zeros((2, 3, 128, 256), np.float32)
        for kk in range(3):
            wT = np.ascontiguousarray(k[:, :, kk].T)
            for kt in range(2):
                arr[kt, kk] = wT[128 * kt:128 * (kt + 1)]
        W[name] = arr.reshape(6, 128, 256)
    W["wfd2"] = tile_k(np.ascontiguousarray(f["w_fd2"].T), 2)        # [2,128,320]

    # fwc with tmp permutation + padding (see emit_step slab layout)
    perm = np.full(256, -1, np.int64)
    perm[0:40] = np.arange(80, 120)      # pred[2:-2]
    perm[40:120] = np.arange(160, 240)   # phs
    perm[128:168] = np.arange(120, 160)  # prevn
    perm[168:248] = np.arange(0, 80)     # c
    wfwcT = f["w_fwc"].T                                             # [720, 256]
    wpad = np.zeros((768, 256), np.float32)
    for q in range(3):
        for i in range(256):
            if perm[i] >= 0:
                wpad[256 * q + i] = wfwcT[240 * q + perm[i]]
    W["wfwc"] = wpad.reshape(6, 128, 256)

    glus = [f["w_fwc_glu"], f["w_sd2"], f["w_sd2_glu"],
            f["w_g1_glu"], f["w_g2_glu"], f["w_g3_glu"]]
    W["wglu"] = np.stack([tile_k(np.ascontiguousarray(w.T), 2) for w in glus]).reshape(12, 128, 256)

    rz, ihn = [], []
    for gi in range(3):
        w_ih, w_hh = f[f"w_g{gi+1}_ih"], f[f"w_g{gi+1}_hh"]
        cat = np.concatenate([w_ih[0:512].T, w_hh[0:512].T], 0)      # [512, 512]
        rz.append(tile_k(np.ascontiguousarray(cat), 4))
        ihn.append(np.stack([tile_k(np.ascontiguousarray(w_ih[512:768].T), 2),
                             tile_k(np.ascontiguousarray(w_hh[512:768].T), 2)]))
    W["wrz"] = np.stack(rz).reshape(12, 128, 512)
    W["wihn"] = np.stack(ihn).reshape(12, 128, 256)

    W["wsig"] = tile_k(np.ascontiguousarray(f["w_sig_out"].T), 8)    # [8,128,40]
    W["wgain"] = tile_k(np.ascontiguousarray(f["w_gain_out"].T), 8)  # [8,128,1]
    W["bg"] = np.asarray(f["b_gain_out"], np.float32).reshape(1, 1)

    # ---------------- per-core tables ----------------
    in_maps = []
    for c in range(NCORE):
        rows = slice(c * BPC, (c + 1) * BPC)
        cxc = np.zeros((84, CW), np.float32)
        cxc[:, :BPC * T] = xc[rows].transpose(2, 1, 0).reshape(84, BPC * T)
        gtab = np.zeros((steps, 16), np.float32)
        gtab[:, 0:8] = gain_s[rows].T
        gtab[:, 8:16] = ginv_s[rows].T
        offsets = (np.arange(BPC)[:, None] * (256 + 40 * steps)
                   + opos[rows]).astype(np.int32)                    # [8, steps]
        m = dict(W)
        m["cx"] = cxc
        m["phs"] = np.ascontiguousarray(
            phs[rows].transpose(1, 2, 0)).astype(np.float32)         # [steps, 80, 8]
        m["gtab"] = gtab
        m["gi8"] = np.ascontiguousarray(ginv_s[rows])                # [8, steps]
        m["offs"] = offsets
        in_maps.append(m)
    return in_maps


# ----------------------------------------------------------------------------
# Device program
# ----------------------------------------------------------------------------

def build_nc(steps=STEPS, chunk=16, debug=False):
    import concourse.bass as bass
    import concourse.mybir as mybir
    import concourse.tile as tile
    from concourse import bacc
    from concourse.bass import ds
    from concourse.masks import make_identity
    from concourse.tile import add_dep_helper

    F32, I32 = mybir.dt.float32, mybir.dt.int32
    AF = mybir.ActivationFunctionType
    OP = mybir.AluOpType

    assert steps % chunk == 0 and chunk % 4 == 0
    bufw = 256 + 40 * steps

    nc = bacc.Bacc(None)
    P = nc.declare_dram_parameter
    cx = P("cx", [84, CW], F32, isOutput=False)
    wfd1 = P("wfd1", [84, 256], F32, isOutput=False)
    k1 = P("k1", [6, 128, 256], F32, isOutput=False)
    k2 = P("k2", [6, 128, 256], F32, isOutput=False)
    wfd2 = P("wfd2", [2, 128, 320], F32, isOutput=False)
    wfwc = P("wfwc", [6, 128, 256], F32, isOutput=False)
    wglu = P("wglu", [12, 128, 256], F32, isOutput=False)
    wrz = P("wrz", [12, 128, 512], F32, isOutput=False)
    wihn = P("wihn", [12, 128, 256], F32, isOutput=False)
    wsig = P("wsig", [8, 128, 40], F32, isOutput=False)
    wgain = P("wgain", [8, 128, 1], F32, isOutput=False)
    bg = P("bg", [1, 1], F32, isOutput=False)
    phs_d = P("phs", [steps, 80, 8], F32, isOutput=False)
    gtab_d = P("gtab", [steps, 16], F32, isOutput=False)
    gi8_d = P("gi8", [8, steps], F32, isOutput=False)
    offs_d = P("offs", [8, steps], I32, isOutput=False)
    buf = P("buf", [8, bufw], F32, isOutput=True)
    if debug:
        dbg = P("dbg", [8, 128, 16], F32, isOutput=True)
        dbgc = P("dbgc", [80, 64], F32, isOutput=True)

    def bcast_part(ap, nparts):
        return bass.AP(ap.tensor, ap.offset, [[0, nparts]] + list(ap.ap))

    with tile.TileContext(nc) as tc:
        with (
            tc.tile_pool(name="wp", bufs=1) as wp,
            tc.tile_pool(name="cnd", bufs=1) as cndp,
            tc.tile_pool(name="st", bufs=1) as stp,
            tc.tile_pool(name="act", bufs=2) as actp,
        ):
            # ------------- resident weights -------------
            def wtile3(n, c, src, nm, pool=wp):
                t = pool.tile([128, n * c], F32, name=nm, tag=nm)
                nc.sync.dma_start(t[:].rearrange("p (a c) -> p a c", a=n),
                                  src.rearrange("a p c -> p a c"))
                return t

            Wfwc = wtile3(6, 256, wfwc[:], "Wfwc")
            Wglu = wtile3(12, 256, wglu[:], "Wglu")
            Wrz = wtile3(12, 512, wrz[:], "Wrz")
            Wihn = wtile3(12, 256, wihn[:], "Wihn")
            Wsig = wtile3(8, 40, wsig[:], "Wsig")
            Wgain = wtile3(8, 1, wgain[:], "Wgain")
            Bg = wp.tile([1, 1], F32)
            nc.sync.dma_start(Bg[:], bg[:])
            ident = wp.tile([128, 128], F32)
            make_identity(nc, ident[:])
            ones40 = wp.tile([1, 40], F32)
            nc.gpsimd.memset(ones40[:], 1.0)

            # ------------- zero the exc history head -------------
            zt = stp.tile([8, 256], F32)
            nc.gpsimd.memset(zt[:], 0.0)
            zero_dma = nc.sync.dma_start(buf[:, 0:256], zt[:])

            # ------------- cond net (scoped pools) -------------
            condT = [cndp.tile([80, CW], F32, tag=f"cs{j}", name=f"condT{j}")
                     for j in range(4)]
            with (
                tc.tile_pool(name="cw", bufs=1) as cwp,
                tc.tile_pool(name="big", bufs=4) as bigp,
                tc.tile_pool(name="psc", bufs=2, space="PSUM") as psc,
            ):
                Wfd1 = cwp.tile([84, 256], F32)
                nc.sync.dma_start(Wfd1[:], wfd1[:])
                Wk1 = wtile3(6, 256, k1[:], "Wk1", pool=cwp)
                Wk2 = wtile3(6, 256, k2[:], "Wk2", pool=cwp)
                Wfd2 = wtile3(2, 320, wfd2[:], "Wfd2", pool=cwp)

                NCHUNKS = [(i * 512, min(512, 3240 - i * 512)) for i in range(7)]

                cxt = bigp.tile([84, CW], F32, tag="big")
                nc.sync.dma_start(cxt[:], cx[:])
                t0 = [bigp.tile([128, CW], F32, tag="big", name=f"t0_{i}")
                      for i in range(2)]
                for t in t0:
                    nc.gpsimd.memset(t[:], 0.0)
                for m in range(2):
                    for c0, cl in NCHUNKS:
                        ps = psc.tile([128, 512], F32, tag="psc")
                        nc.tensor.matmul(ps[:, :cl], Wfd1[:, 128 * m:128 * (m + 1)],
                                         cxt[:, c0:c0 + cl], start=True, stop=True)
                        nc.scalar.activation(t0[m][:, c0:c0 + cl], ps[:, :cl],
                                             AF.Tanh)

                def conv_layer(src, Wk, dst):
                    for m in range(2):
                        for c0, cl in NCHUNKS:
                            ps = psc.tile([128, 512], F32, tag="psc")
                            n = 0
                            for kt in range(2):
                                for kk in range(3):
                                    base = (kt * 3 + kk) * 256 + 128 * m
                                    nc.tensor.matmul(
                                        ps[:, :cl], Wk[:, base:base + 128],
                                        src[kt][:, c0 + 8 * kk:c0 + 8 * kk + cl],
                                        start=(n == 0), stop=(n == 5))
                                    n += 1
                            nc.scalar.activation(dst[m][:, c0:c0 + cl], ps[:, :cl],
                                                 AF.Tanh)

                t1c = [bigp.tile([128, CW], F32, tag="big", name=f"t1c_{i}")
                       for i in range(2)]
                for t in t1c:
                    nc.gpsimd.memset(t[:], 0.0)
                conv_layer(t0, Wk1, t1c)
                t2c = [bigp.tile([128, CW], F32, tag="big", name=f"t2c_{i}")
                       for i in range(2)]
                for t in t2c:
                    nc.gpsimd.memset(t[:], 0.0)
                conv_layer(t1c, Wk2, t2c)

                # fd2 -> condT[j] [80, CW] (t-major cols: frame f at 8f)
                for j in range(4):
                    for c0, cl in NCHUNKS:
                        ps = psc.tile([128, 512], F32, tag="psc")
                        for kt in range(2):
                            base = kt * 320 + 80 * j
                            nc.tensor.matmul(ps[:80, :cl], Wfd2[:, base:base + 80],
                                             t2c[kt][:, c0:c0 + cl],
                                             start=(kt == 0), stop=(kt == 1))
                        nc.scalar.activation(condT[j][:, c0:c0 + cl], ps[:80, :cl],
                                             AF.Tanh)

            if debug:
                nc.sync.dma_start(dbgc[:], condT[0][:, 0:64])

            # ------------- recurrence state -------------
            slab = stp.tile([128, 8 * 16], F32)      # 8 slots x 16 cols
            nc.gpsimd.memset(slab[:], 0.0)
            h = [stp.tile([128, 16], F32, name=f"h{i}") for i in range(3)]
            for t in h:
                nc.gpsimd.memset(t[:], 0.0)
            sigT = stp.tile([40, 8], F32)
            nc.gpsimd.memset(sigT[:], 0.0)
            gvec = [stp.tile([128, 16], F32, name=f"gv{i}") for i in range(3)]
            d2t = stp.tile([128, 16], F32)

            gb_st = stp.tile([40, chunk * 16], F32)
            gi_st = stp.tile([8, chunk], F32)
            offs_st = stp.tile([8, chunk], I32)

            last_sig_dma = [zero_dma]

            with (
                tc.tile_pool(name="psfwc", bufs=2, space="PSUM") as psfwc,
                tc.tile_pool(name="psglu", bufs=1, space="PSUM") as psglu,
                tc.tile_pool(name="psrz", bufs=1, space="PSUM") as psrz,
                tc.tile_pool(name="psin", bufs=1, space="PSUM") as psin,
                tc.tile_pool(name="pshn", bufs=1, space="PSUM") as pshn,
                tc.tile_pool(name="pssm", bufs=2, space="PSUM") as pssm,
            ):

                def emit_step(ci, j):
                    w = (j - 2) % 4          # chunk % 4 == 0 -> s%4 == j%4
                    sa, sb = j % 4, j % 4 + 4

                    # ---- gather pred ----
                    gth = actp.tile([8, 44], F32, tag="gth")
                    g_i = nc.gpsimd.indirect_dma_start(
                        out=gth[:], out_offset=None, in_=buf[:, :],
                        in_offset=bass.IndirectOffsetOnAxis(
                            ap=offs_st[:, j:j + 1], axis=1))
                    add_dep_helper(g_i.ins, last_sig_dma[0].ins,
                                   reason="gather after sig write")
                    gth_s = actp.tile([8, 44], F32, tag="gths")
                    nc.vector.tensor_scalar(gth_s[:], gth[:], gi_st[:, j:j + 1],
                                            None, OP.mult)
                    pps = pssm.tile([48, 128], F32, tag="sm")
                    nc.tensor.transpose(out=pps[0:40, 0:8], in_=gth_s[:, 2:42],
                                        identity=ident[:8, :8])
                    # ---- slab writes ----
                    nc.scalar.copy(slab[0:40, 16 * sa:16 * sa + 8], pps[0:40, 0:8])
                    nc.vector.tensor_copy(slab[0:40, 16 * sb:16 * sb + 8],
                                          pps[0:40, 0:8])
                    for sl in (sa, sb):
                        nc.vector.tensor_tensor(
                            out=slab[0:40, 16 * sl + 8:16 * sl + 16],
                            in0=sigT[:], in1=gb_st[:, 16 * j + 8:16 * j + 16],
                            op=OP.mult)
                    if ci is None:
                        pv = phs_d[j:j + 1, :, :]
                    else:
                        pv = phs_d[ds(ci * chunk + j, 1), :, :]
                    pv = pv.rearrange("one p b -> p (one b)")
                    fidx = (j // 4) if ci is None else ci * (chunk // 4) + j // 4
                    for sl in (sa, sb):
                        nc.sync.dma_start(slab[40:120, 16 * sl:16 * sl + 8], pv)
                        nc.sync.dma_start(
                            slab[40:120, 16 * sl + 8:16 * sl + 16],
                            condT[j % 4][:, ds(fidx * 8, 8)])

                    # ---- fwc + glu ----
                    pf = psfwc.tile([128, 16], F32, tag="fwc")
                    for m in range(2):
                        for t in range(6):
                            q, hh = t // 2, t % 2
                            rhs = slab[:, 16 * (w + q) + 8 * hh:
                                       16 * (w + q) + 8 * hh + 8]
                            nc.tensor.matmul(pf[:, 8 * m:8 * m + 8],
                                             Wfwc[:, 256 * t + 128 * m:
                                                  256 * t + 128 * (m + 1)],
                                             rhs, start=(t == 0), stop=(t == 5))
                    t1 = actp.tile([128, 16], F32, tag="t1")
                    nc.scalar.activation(t1[:], pf[:], AF.Tanh)

                    def glu_mm(xin, li):
                        pg = psglu.tile([128, 16], F32, tag="glu")
                        for m in range(2):
                            for kt in range(2):
                                base = (li * 2 + kt) * 256 + 128 * m
                                nc.tensor.matmul(pg[:, 8 * m:8 * m + 8],
                                                 Wglu[:, base:base + 128],
                                                 xin[:, 8 * kt:8 * kt + 8],
                                                 start=(kt == 0), stop=(kt == 1))
                        return pg

                    pg1 = glu_mm(t1, 0)
                    sg1 = actp.tile([128, 16], F32, tag="sg")
                    nc.scalar.activation(sg1[:], pg1[:], AF.Sigmoid)
                    fout = actp.tile([128, 16], F32, tag="fout")
                    nc.vector.tensor_tensor(out=fout[:], in0=t1[:], in1=sg1[:],
                                            op=OP.mult)
                    pg2 = glu_mm(fout, 1)
                    t2 = actp.tile([128, 16], F32, tag="t2")
                    nc.scalar.activation(t2[:], pg2[:], AF.Tanh)
                    pg3 = glu_mm(t2, 2)
                    sg2 = actp.tile([128, 16], F32, tag="sg2")
                    nc.scalar.activation(sg2[:], pg3[:], AF.Sigmoid)
                    nc.vector.tensor_tensor(out=d2t[:], in0=t2[:], in1=sg2[:],
                                            op=OP.mult)

                    # ---- GRUs ----
                    xin = d2t
                    for gi in range(3):
                        prz = psrz.tile([128, 32], F32, tag="rz")
                        for m in range(4):
                            for kt in range(4):
                                rhs = (xin if kt < 2 else h[gi])[
                                    :, 8 * (kt % 2):8 * (kt % 2) + 8]
                                base = (gi * 4 + kt) * 512 + 128 * m
                                nc.tensor.matmul(prz[:, 8 * m:8 * m + 8],
                                                 Wrz[:, base:base + 128],
                                                 rhs, start=(kt == 0),
                                                 stop=(kt == 3))
                        pin = psin.tile([128, 16], F32, tag="inn")
                        phn = pshn.tile([128, 16], F32, tag="hn")
                        for m in range(2):
                            for kt in range(2):
                                basei = (gi * 4 + kt) * 256 + 128 * m
                                baseh = (gi * 4 + 2 + kt) * 256 + 128 * m
                                nc.tensor.matmul(pin[:, 8 * m:8 * m + 8],
                                                 Wihn[:, basei:basei + 128],
                                                 xin[:, 8 * kt:8 * kt + 8],
                                                 start=(kt == 0), stop=(kt == 1))
                                nc.tensor.matmul(phn[:, 8 * m:8 * m + 8],
                                                 Wihn[:, baseh:baseh + 128],
                                                 h[gi][:, 8 * kt:8 * kt + 8],
                                                 start=(kt == 0), stop=(kt == 1))
                        rzs = actp.tile([128, 32], F32, tag="rzs")
                        nc.scalar.activation(rzs[:], prz[:], AF.Sigmoid)
                        tt = actp.tile([128, 16], F32, tag="tt")
                        nc.vector.tensor_tensor(out=tt[:], in0=rzs[:, 0:16],
                                                in1=phn[:], op=OP.mult)
                        pren = actp.tile([128, 16], F32, tag="pren")
                        nc.vector.tensor_tensor(out=pren[:], in0=pin[:],
                                                in1=tt[:], op=OP.add)
                        nt = actp.tile([128, 16], F32, tag="nt")
                        nc.scalar.activation(nt[:], pren[:], AF.Tanh)
                        hmn = actp.tile([128, 16], F32, tag="hmn")
                        nc.vector.tensor_tensor(out=hmn[:], in0=h[gi][:],
                                                in1=nt[:], op=OP.subtract)
                        zh = actp.tile([128, 16], F32, tag="zh")
                        nc.vector.tensor_tensor(out=zh[:], in0=rzs[:, 16:32],
                                                in1=hmn[:], op=OP.mult)
                        nc.vector.tensor_tensor(out=h[gi][:], in0=nt[:],
                                                in1=zh[:], op=OP.add)
                        pgl = glu_mm(h[gi], 3 + gi)
                        sgl = actp.tile([128, 16], F32, tag="sgl")
                        nc.scalar.activation(sgl[:], pgl[:], AF.Sigmoid)
                        nc.vector.tensor_tensor(out=gvec[gi][:], in0=h[gi][:],
                                                in1=sgl[:], op=OP.mult)
                        xin = gvec[gi]

                    # ---- output heads ----
                    po = pssm.tile([48, 128], F32, tag="sm")
                    cats = [gvec[0], gvec[1], gvec[2], d2t]
                    for kt in range(8):
                        src = cats[kt // 2][:, 8 * (kt % 2):8 * (kt % 2) + 8]
                        nc.tensor.matmul(po[0:40, 16:24],
                                         Wsig[:, 40 * kt:40 * (kt + 1)], src,
                                         start=(kt == 0), stop=(kt == 7))
                    for kt in range(8):
                        src = cats[kt // 2][:, 8 * (kt % 2):8 * (kt % 2) + 8]
                        nc.tensor.matmul(po[0:1, 24:32], Wgain[:, kt:kt + 1], src,
                                         start=(kt == 0), stop=(kt == 7))
                    sigt = actp.tile([40, 8], F32, tag="sigt")
                    nc.scalar.activation(sigt[:], po[0:40, 16:24], AF.Tanh)
                    pgw = actp.tile([1, 32], F32, tag="pgw")
                    nc.scalar.activation(pgw[:, 0:8], po[0:1, 24:32], AF.Sigmoid,
                                         bias=Bg[:, 0:1])
                    nc.vector.tensor_scalar(pgw[:, 8:16], pgw[:, 0:8], -1.0, 1.0,
                                            OP.mult, OP.add)
                    nc.vector.reciprocal(pgw[:, 16:24], pgw[:, 8:16])
                    nc.vector.tensor_tensor(out=pgw[:, 24:32], in0=pgw[:, 0:8],
                                            in1=pgw[:, 16:24], op=OP.mult)
                    ppg = pssm.tile([48, 128], F32, tag="sm")
                    nc.tensor.matmul(ppg[0:40, 32:40], ones40[:], pgw[:, 24:32],
                                     start=True, stop=True)
                    t4 = actp.tile([40, 8], F32, tag="t4")
                    nc.vector.tensor_tensor(out=t4[:], in0=ppg[0:40, 32:40],
                                            in1=slab[0:40, 16 * sa:16 * sa + 8],
                                            op=OP.mult)
                    t5 = actp.tile([40, 8], F32, tag="t5")
                    nc.vector.tensor_tensor(out=t5[:], in0=t4[:], in1=sigt[:],
                                            op=OP.add)
                    nc.vector.tensor_tensor(out=sigT[:], in0=t5[:],
                                            in1=gb_st[:, 16 * j:16 * j + 8],
                                            op=OP.mult)
                    pst = pssm.tile([48, 128], F32, tag="sm")
                    nc.tensor.transpose(out=pst[0:8, 64:104], in_=sigT[:],
                                        identity=ident[:40, :40])
                    sig_sb = actp.tile([8, 40], F32, tag="sigsb")
                    nc.scalar.copy(sig_sb[:], pst[0:8, 64:104])
                    if ci is None:
                        dst = buf[:, 256 + 40 * j:296 + 40 * j]
                    else:
                        dst = buf[:, ds(ci * (40 * chunk) + 40 * j + 256, 40)]
                    sd = nc.sync.dma_start(dst, sig_sb[:])
                    last_sig_dma[0] = sd
                    if debug and ci is None and j == 0:
                        for di, tt_ in enumerate([t1, fout, d2t, h[0], gvec[0],
                                                  gvec[2]]):
                            nc.sync.dma_start(dbg[di][:, 0:16], tt_[:, 0:16])
                        nc.sync.dma_start(dbg[6][0:40, 0:8], sigT[:])
                        nc.sync.dma_start(dbg[7][0:128, 0:16], slab[:, 0:16])

                def emit_chunk(ci):
                    if ci is None:
                        gt_src = gtab_d[0:chunk, :]
                        gi_src = gi8_d[:, 0:chunk]
                        off_src = offs_d[:, 0:chunk]
                    else:
                        gt_src = gtab_d[ds(ci * chunk, chunk), :]
                        gi_src = gi8_d[:, ds(ci * chunk, chunk)]
                        off_src = offs_d[:, ds(ci * chunk, chunk)]
                    nc.sync.dma_start(gb_st[:], bcast_part(gt_src, 40))
                    nc.sync.dma_start(gi_st[:], gi_src)
                    nc.sync.dma_start(offs_st[:], off_src)
                    for j in range(chunk):
                        emit_step(ci, j)

                nchunks = steps // chunk
                if nchunks == 1:
                    emit_chunk(None)
                else:
                    with tc.For_i(0, nchunks, 1,
                                  hint_engines=(mybir.EngineType.PE,
                                                mybir.EngineType.Activation,
                                                mybir.EngineType.DVE,
                                                mybir.EngineType.Pool,
                                                mybir.EngineType.SP)) as ci:
                        emit_chunk(ci)

    nc.compile()
    orig = nc.to_json_bytes
    nc.to_json_bytes = lambda: _legalize_bir_json(orig())
    return nc


_NC_CACHE = {}


def kernel(**inputs):
    from concourse.bass_utils import run_bass_kernel_spmd

    key = (STEPS, 16)
    if key not in _NC_CACHE:
        _NC_CACHE[key] = build_nc(STEPS, 16)
    nc = _NC_CACHE[key]
    in_maps = host_prep(inputs, STEPS)
    res = run_bass_kernel_spmd(nc, in_maps, list(range(NCORE)))
    out = np.concatenate([res.results[c]["buf"][:, 256:] for c in range(NCORE)], 0)
    return out.astype(np.float32)


# revision 18
# speedup vs baseline: 94.9813x; 94.9813x over previous
"""FARGAN vocoder Trainium2 Bass kernel.

Strategy: pure data parallelism — batch 64 sharded 8 rows/core across 8
NeuronCores. Each core runs: (1) the conv cond-net as big matmuls, (2) the
1600-step subframe recurrence with feature-major activations (weights
stationary on the PE), with the pitch-predictor gather done via indirect DMA
from the DRAM output buffer (which doubles as the excitation history).

Host precomputes (numpy/jax-cpu, exact f32 semantics): phase embeddings,
gains, gather offsets, cond-net input, and packs/transposes/pads weights.
"""
import sys, os, json

sys.path.insert(0, "/opt/trn_rl_repo")

import numpy as np

SUB, NSUB, FRAME, CND, FDIM, PEMB = 40, 4, 160, 256, 20, 64
FWC_IN = 240
B, T, NBF = 64, 404, 400
STEPS = NBF * NSUB            # 1600
NCORE, BPC = 8, 8             # cores, batch per core
CHUNK = 16                    # steps per For_i iteration
CW = 3264                     # padded cond-net width (8*404 = 3232 + pad)

# ----------------------------------------------------------------------------
# BIR legalizer: this walrus build allows at most ONE sync wait per
# instruction; hoist extra waits onto same-engine NoOps placed just before.
# ----------------------------------------------------------------------------

def _legalize_bir_json(raw: bytes) -> bytes:
    d = json.loads(raw)
    ctr = [0]

    def mk_nop(engine, wait, debug):
        ctr[0] += 1
        return {"debug": debug, "engine": engine, "ins": [],
                "name": f"legal-nop-{ctr[0]}", "opcode": "NoOp", "outs": [],
                "sync_info": {"on_update": [], "on_wait": [wait]}}

    for f in d.get("functions", []):
        for b in f.get("blocks", []):
            out = []
            for inst in b["instructions"]:
                si = inst.get("sync_info")
                waits = (si or {}).get("on_wait") or []
                if len(waits) > 1:
                    for w in waits[:-1]:
                        out.append(mk_nop(inst["engine"], w, inst.get("debug", 0)))
                    si["on_wait"] = [waits[-1]]
                out.append(inst)
            b["instructions"] = out
    return json.dumps(d).encode()


# ----------------------------------------------------------------------------
# Host precompute
# ----------------------------------------------------------------------------

def _phase_embedding(period):
    """preal, pimag [B, STEPS*SUB] — computed with jax CPU, matching reference."""
    import jax, jax.numpy as jnp
    cpu = jax.devices("cpu")[0]
    with jax.default_device(cpu):
        periods = jnp.asarray(period[:, 3:-1]).astype(jnp.float32)
        nB = periods.shape[0]
        w0 = 2.0 * jnp.pi / periods
        shift = 2.0 * jnp.pi * jax.random.uniform(
            jax.random.key(42), (nB, 1), periods.dtype) / FRAME
        w0s = jnp.concatenate([shift, w0[:, :-1]], 1)
        cum = FRAME * jnp.cumsum(w0s, 1)
        fine = w0[:, :, None] * jnp.arange(FRAME, dtype=w0.dtype)
        emb = (cum[:, :, None] + fine).reshape(nB, -1)
        return np.asarray(jnp.cos(emb)), np.asarray(jnp.sin(emb))


def host_prep(inputs, steps=STEPS):
    """Returns list of per-core input maps."""
    f = {k: np.asarray(v) for k, v in inputs.items() if hasattr(v, "shape")}
    features, period = np.asarray(f["features"], np.float32), np.asarray(f["period"])
    nbf = steps // NSUB

    preal, pimag = _phase_embedding(period)
    pr = preal.reshape(B, STEPS, SUB)[:, :steps]
    pi = pimag.reshape(B, STEPS, SUB)[:, :steps]
    phs = np.concatenate([pr, pi], -1)                       # [B, steps, 80]

    gain = np.float32(0.03) * np.power(
        np.float32(10.0),
        np.float32(0.5) * features[:, 3:3 + nbf, 0] / np.float32(np.sqrt(18.0)))
    gain_s = np.repeat(gain, NSUB, axis=1).astype(np.float32)        # [B, steps]
    ginv_s = (np.float32(1.0) / (np.float32(1e-5) + gain_s)).astype(np.float32)

    pit = np.clip(period[:, 3:3 + nbf], SUB + 2, 254)
    o = (254 - pit).astype(np.int64)
    s_idx = np.arange(steps)
    opos = 40 * s_idx[None, :] + np.repeat(o, NSUB, axis=1)          # [B, steps]

    p_emb = f["pembed"][period]                                      # [B, T, 64]
    xc = np.concatenate([features, p_emb], -1).astype(np.float32)    # [B, T, 84]

    # ---------------- weights (shared across cores) ----------------
    W = {}
    W["wfd1"] = np.ascontiguousarray(f["w_fd1"].T).astype(np.float32)

    def tile_k(wT, ktiles):
        K, M = wT.shape
        out = np.zeros((ktiles, 128, M), np.float32)
        for t in range(ktiles):
            blk = wT[128 * t:128 * (t + 1)]
            out[t, :blk.shape[0]] = blk
        return out

    for name, k in (("k1", f["k_fc1"]), ("k2", f["k_fc2"])):
        arr = np.zeros((2, 3, 128, 256), np.float32)
        for kk in range(3):
            wT = np.ascontiguousarray(k[:, :, kk].T)
            for kt in range(2):
                arr[kt, kk] = wT[128 * kt:128 * (kt + 1)]
        W[name] = arr.reshape(6, 128, 256)
    W["wfd2"] = tile_k(np.ascontiguousarray(f["w_fd2"].T), 2)        # [2,128,320]

    # fwc with tmp permutation + padding (see emit_step slab layout)
    perm = np.full(256, -1, np.int64)
    perm[0:40] = np.arange(80, 120)      # pred[2:-2]
    perm[40:120] = np.arange(160, 240)   # phs
    perm[128:168] = np.arange(120, 160)  # prevn
    perm[168:248] = np.arange(0, 80)     # c
    wfwcT = f["w_fwc"].T                                             # [720, 256]
    wpad = np.zeros((768, 256), np.float32)
    for q in range(3):
        for i in range(256):
            if perm[i] >= 0:
                wpad[256 * q + i] = wfwcT[240 * q + perm[i]]
    W["wfwc"] = wpad.reshape(6, 128, 256)

    glus = [f["w_fwc_glu"], f["w_sd2"], f["w_sd2_glu"],
            f["w_g1_glu"], f["w_g2_glu"], f["w_g3_glu"]]
    W["wglu"] = np.stack([tile_k(np.ascontiguousarray(w.T), 2) for w in glus]).reshape(12, 128, 256)

    rz, ihn = [], []
    for gi in range(3):
        w_ih, w_hh = f[f"w_g{gi+1}_ih"], f[f"w_g{gi+1}_hh"]
        cat = np.concatenate([w_ih[0:512].T, w_hh[0:512].T], 0)      # [512, 512]
        rz.append(tile_k(np.ascontiguousarray(cat), 4))
        ihn.append(np.stack([tile_k(np.ascontiguousarray(w_ih[512:768].T), 2),
                             tile_k(np.ascontiguousarray(w_hh[512:768].T), 2)]))
    W["wrz"] = np.stack(rz).reshape(12, 128, 512)
    W["wihn"] = np.stack(ihn).reshape(12, 128, 256)

    W["wsig"] = tile_k(np.ascontiguousarray(f["w_sig_out"].T), 8)    # [8,128,40]
    W["wgain"] = tile_k(np.ascontiguousarray(f["w_gain_out"].T), 8)  # [8,128,1]
    W["bg"] = np.asarray(f["b_gain_out"], np.float32).reshape(1, 1)

    # ---------------- per-core tables ----------------
    in_maps = []
    for c in range(NCORE):
        rows = slice(c * BPC, (c + 1) * BPC)
        cxc = np.zeros((84, CW), np.float32)
        cxc[:, :BPC * T] = xc[rows].transpose(2, 1, 0).reshape(84, BPC * T)
        gtab = np.zeros((steps, 16), np.float32)
        gtab[:, 0:8] = gain_s[rows].T
        gtab[:, 8:16] = ginv_s[rows].T
        offsets = (np.arange(BPC)[:, None] * (256 + 40 * steps)
                   + opos[rows]).astype(np.int32)                    # [8, steps]
        m = dict(W)
        m["cx"] = cxc
        m["phs"] = np.ascontiguousarray(
            phs[rows].transpose(1, 2, 0)).astype(np.float32)         # [steps, 80, 8]
        m["gtab"] = gtab
        m["gi8"] = np.ascontiguousarray(ginv_s[rows])                # [8, steps]
        m["offs"] = offsets
        in_maps.append(m)
    return in_maps


# ----------------------------------------------------------------------------
# Device program
# ----------------------------------------------------------------------------

def build_nc(steps=STEPS, chunk=16, debug=False, run_chunks=None, repeats=1, sreset=False, hints=True):
    import concourse.bass as bass
    import concourse.mybir as mybir
    import concourse.tile as tile
    from concourse import bacc
    from concourse.bass import ds
    from concourse.masks import make_identity
    from concourse.tile import add_dep_helper

    F32, I32 = mybir.dt.float32, mybir.dt.int32
    AF = mybir.ActivationFunctionType
    OP = mybir.AluOpType

    assert steps % chunk == 0 and chunk % 4 == 0
    bufw = 256 + 40 * steps

    nc = bacc.Bacc(None)
    P = nc.declare_dram_parameter
    cx = P("cx", [84, CW], F32, isOutput=False)
    wfd1 = P("wfd1", [84, 256], F32, isOutput=False)
    k1 = P("k1", [6, 128, 256], F32, isOutput=False)
    k2 = P("k2", [6, 128, 256], F32, isOutput=False)
    wfd2 = P("wfd2", [2, 128, 320], F32, isOutput=False)
    wfwc = P("wfwc", [6, 128, 256], F32, isOutput=False)
    wglu = P("wglu", [12, 128, 256], F32, isOutput=False)
    wrz = P("wrz", [12, 128, 512], F32, isOutput=False)
    wihn = P("wihn", [12, 128, 256], F32, isOutput=False)
    wsig = P("wsig", [8, 128, 40], F32, isOutput=False)
    wgain = P("wgain", [8, 128, 1], F32, isOutput=False)
    bg = P("bg", [1, 1], F32, isOutput=False)
    phs_d = P("phs", [steps, 80, 8], F32, isOutput=False)
    gtab_d = P("gtab", [steps, 16], F32, isOutput=False)
    gi8_d = P("gi8", [8, steps], F32, isOutput=False)
    offs_d = P("offs", [8, steps], I32, isOutput=False)
    buf = P("buf", [8, bufw], F32, isOutput=True)
    if debug:
        dbg = P("dbg", [8, 128, 16], F32, isOutput=True)
        dbgc = P("dbgc", [80, 64], F32, isOutput=True)

    def bcast_part(ap, nparts):
        return bass.AP(ap.tensor, ap.offset, [[0, nparts]] + list(ap.ap))

    with tile.TileContext(nc) as tc:
        with (
            tc.tile_pool(name="wp", bufs=1) as wp,
            tc.tile_pool(name="cnd", bufs=1) as cndp,
            tc.tile_pool(name="st", bufs=1) as stp,
            tc.tile_pool(name="act", bufs=2) as actp,
        ):
            # ------------- resident weights -------------
            def wtile3(n, c, src, nm, pool=wp):
                t = pool.tile([128, n * c], F32, name=nm, tag=nm)
                nc.sync.dma_start(t[:].rearrange("p (a c) -> p a c", a=n),
                                  src.rearrange("a p c -> p a c"))
                return t

            Wfwc = wtile3(6, 256, wfwc[:], "Wfwc")
            Wglu = wtile3(12, 256, wglu[:], "Wglu")
            Wrz = wtile3(12, 512, wrz[:], "Wrz")
            Wihn = wtile3(12, 256, wihn[:], "Wihn")
            Wsig = wtile3(8, 40, wsig[:], "Wsig")
            Wgain = wtile3(8, 1, wgain[:], "Wgain")
            Bg = wp.tile([1, 1], F32)
            nc.sync.dma_start(Bg[:], bg[:])
            ident = wp.tile([128, 128], F32)
            make_identity(nc, ident[:])
            ones40 = wp.tile([1, 40], F32)
            nc.gpsimd.memset(ones40[:], 1.0)

            # ------------- zero the exc history head -------------
            zt = stp.tile([8, 256], F32)
            nc.gpsimd.memset(zt[:], 0.0)
            zero_dma = nc.sync.dma_start(buf[:, 0:256], zt[:])

            # ------------- cond net (scoped pools) -------------
            condT = [cndp.tile([80, CW], F32, tag=f"cs{j}", name=f"condT{j}")
                     for j in range(4)]
            with (
                tc.tile_pool(name="cw", bufs=1) as cwp,
                tc.tile_pool(name="big", bufs=4) as bigp,
                tc.tile_pool(name="psc", bufs=2, space="PSUM") as psc,
            ):
                Wfd1 = cwp.tile([84, 256], F32)
                nc.sync.dma_start(Wfd1[:], wfd1[:])
                Wk1 = wtile3(6, 256, k1[:], "Wk1", pool=cwp)
                Wk2 = wtile3(6, 256, k2[:], "Wk2", pool=cwp)
                Wfd2 = wtile3(2, 320, wfd2[:], "Wfd2", pool=cwp)

                NCHUNKS = [(i * 512, min(512, 3240 - i * 512)) for i in range(7)]

                cxt = bigp.tile([84, CW], F32, tag="big")
                nc.sync.dma_start(cxt[:], cx[:])
                t0 = [bigp.tile([128, CW], F32, tag="big", name=f"t0_{i}")
                      for i in range(2)]
                for t in t0:
                    nc.gpsimd.memset(t[:], 0.0)
                for m in range(2):
                    for c0, cl in NCHUNKS:
                        ps = psc.tile([128, 512], F32, tag="psc")
                        nc.tensor.matmul(ps[:, :cl], Wfd1[:, 128 * m:128 * (m + 1)],
                                         cxt[:, c0:c0 + cl], start=True, stop=True)
                        nc.scalar.activation(t0[m][:, c0:c0 + cl], ps[:, :cl],
                                             AF.Tanh)

                def conv_layer(src, Wk, dst):
                    for m in range(2):
                        for c0, cl in NCHUNKS:
                            ps = psc.tile([128, 512], F32, tag="psc")
                            n = 0
                            for kt in range(2):
                                for kk in range(3):
                                    base = (kt * 3 + kk) * 256 + 128 * m
                                    nc.tensor.matmul(
                                        ps[:, :cl], Wk[:, base:base + 128],
                                        src[kt][:, c0 + 8 * kk:c0 + 8 * kk + cl],
                                        start=(n == 0), stop=(n == 5))
                                    n += 1
                            nc.scalar.activation(dst[m][:, c0:c0 + cl], ps[:, :cl],
                                                 AF.Tanh)

                t1c = [bigp.tile([128, CW], F32, tag="big", name=f"t1c_{i}")
                       for i in range(2)]
                for t in t1c:
                    nc.gpsimd.memset(t[:], 0.0)
                conv_layer(t0, Wk1, t1c)
                t2c = [bigp.tile([128, CW], F32, tag="big", name=f"t2c_{i}")
                       for i in range(2)]
                for t in t2c:
                    nc.gpsimd.memset(t[:], 0.0)
                conv_layer(t1c, Wk2, t2c)

                # fd2 -> condT[j] [80, CW] (t-major cols: frame f at 8f)
                for j in range(4):
                    for c0, cl in NCHUNKS:
                        ps = psc.tile([128, 512], F32, tag="psc")
                        for kt in range(2):
                            base = kt * 320 + 80 * j
                            nc.tensor.matmul(ps[:80, :cl], Wfd2[:, base:base + 80],
                                             t2c[kt][:, c0:c0 + cl],
                                             start=(kt == 0), stop=(kt == 1))
                        nc.scalar.activation(condT[j][:, c0:c0 + cl], ps[:80, :cl],
                                             AF.Tanh)

            if debug:
                nc.sync.dma_start(dbgc[:], condT[0][:, 0:64])

            # ------------- recurrence state -------------
            slab = stp.tile([128, 8 * 16], F32)      # 8 slots x 16 cols
            nc.gpsimd.memset(slab[:], 0.0)
            h = [stp.tile([128, 16], F32, name=f"h{i}") for i in range(3)]
            for t in h:
                nc.gpsimd.memset(t[:], 0.0)
            sigT = stp.tile([40, 8], F32)
            nc.gpsimd.memset(sigT[:], 0.0)
            gvec = [stp.tile([128, 16], F32, name=f"gv{i}") for i in range(3)]
            d2t = stp.tile([128, 16], F32)

            gb_st = stp.tile([40, chunk * 16], F32)
            gi_st = stp.tile([8, chunk], F32)
            offs_st = stp.tile([8, chunk], I32)

            last_sig_dma = [zero_dma]

            with (
                tc.tile_pool(name="psfwc", bufs=2, space="PSUM") as psfwc,
                tc.tile_pool(name="psglu", bufs=1, space="PSUM") as psglu,
                tc.tile_pool(name="psrz", bufs=1, space="PSUM") as psrz,
                tc.tile_pool(name="psin", bufs=1, space="PSUM") as psin,
                tc.tile_pool(name="pshn", bufs=1, space="PSUM") as pshn,
                tc.tile_pool(name="pssm", bufs=2, space="PSUM") as pssm,
            ):

                def emit_step(ci, j):
                    w = (j - 2) % 4          # chunk % 4 == 0 -> s%4 == j%4
                    sa, sb = j % 4, j % 4 + 4

                    # ---- gather pred ----
                    gth = actp.tile([8, 44], F32, tag="gth")
                    g_i = nc.gpsimd.indirect_dma_start(
                        out=gth[:], out_offset=None, in_=buf[:, :],
                        in_offset=bass.IndirectOffsetOnAxis(
                            ap=offs_st[:, j:j + 1], axis=1))
                    add_dep_helper(g_i.ins, last_sig_dma[0].ins,
                                   reason="gather after sig write")
                    gth_s = actp.tile([8, 44], F32, tag="gths")
                    nc.vector.tensor_scalar(gth_s[:], gth[:], gi_st[:, j:j + 1],
                                            None, OP.mult)
                    pps = pssm.tile([48, 128], F32, tag="sm")
                    nc.tensor.transpose(out=pps[0:40, 0:8], in_=gth_s[:, 2:42],
                                        identity=ident[:8, :8])
                    # ---- slab writes ----
                    nc.scalar.copy(slab[0:40, 16 * sa:16 * sa + 8], pps[0:40, 0:8])
                    nc.vector.tensor_copy(slab[0:40, 16 * sb:16 * sb + 8],
                                          pps[0:40, 0:8])
                    for sl in (sa, sb):
                        nc.vector.tensor_tensor(
                            out=slab[0:40, 16 * sl + 8:16 * sl + 16],
                            in0=sigT[:], in1=gb_st[:, 16 * j + 8:16 * j + 16],
                            op=OP.mult)
                    if ci is None:
                        pv = phs_d[j:j + 1, :, :]
                    else:
                        pv = phs_d[ds(ci * chunk + j, 1), :, :]
                    pv = pv.rearrange("one p b -> p (one b)")
                    fidx = (j // 4) if ci is None else ci * (chunk // 4) + j // 4
                    for sl in (sa, sb):
                        nc.sync.dma_start(slab[40:120, 16 * sl:16 * sl + 8], pv)
                        nc.sync.dma_start(
                            slab[40:120, 16 * sl + 8:16 * sl + 16],
                            condT[j % 4][:, ds(fidx * 8, 8)])

                    # ---- fwc + glu ----
                    pf = psfwc.tile([128, 16], F32, tag="fwc")
                    for m in range(2):
                        for t in range(6):
                            q, hh = t // 2, t % 2
                            rhs = slab[:, 16 * (w + q) + 8 * hh:
                                       16 * (w + q) + 8 * hh + 8]
                            nc.tensor.matmul(pf[:, 8 * m:8 * m + 8],
                                             Wfwc[:, 256 * t + 128 * m:
                                                  256 * t + 128 * (m + 1)],
                                             rhs, start=(t == 0), stop=(t == 5))
                    t1 = actp.tile([128, 16], F32, tag="t1")
                    nc.scalar.activation(t1[:], pf[:], AF.Tanh)

                    def glu_mm(xin, li):
                        pg = psglu.tile([128, 16], F32, tag="glu")
                        for m in range(2):
                            for kt in range(2):
                                base = (li * 2 + kt) * 256 + 128 * m
                                nc.tensor.matmul(pg[:, 8 * m:8 * m + 8],
                                                 Wglu[:, base:base + 128],
                                                 xin[:, 8 * kt:8 * kt + 8],
                                                 start=(kt == 0), stop=(kt == 1))
                        return pg

                    pg1 = glu_mm(t1, 0)
                    sg1 = actp.tile([128, 16], F32, tag="sg")
                    nc.scalar.activation(sg1[:], pg1[:], AF.Sigmoid)
                    fout = actp.tile([128, 16], F32, tag="fout")
                    nc.vector.tensor_tensor(out=fout[:], in0=t1[:], in1=sg1[:],
                                            op=OP.mult)
                    pg2 = glu_mm(fout, 1)
                    t2 = actp.tile([128, 16], F32, tag="t2")
                    nc.scalar.activation(t2[:], pg2[:], AF.Tanh)
                    pg3 = glu_mm(t2, 2)
                    sg2 = actp.tile([128, 16], F32, tag="sg2")
                    nc.scalar.activation(sg2[:], pg3[:], AF.Sigmoid)
                    nc.vector.tensor_tensor(out=d2t[:], in0=t2[:], in1=sg2[:],
                                            op=OP.mult)

                    # ---- GRUs ----
                    xin = d2t
                    for gi in range(3):
                        prz = psrz.tile([128, 32], F32, tag="rz")
                        for m in range(4):
                            for kt in range(4):
                                rhs = (xin if kt < 2 else h[gi])[
                                    :, 8 * (kt % 2):8 * (kt % 2) + 8]
                                base = (gi * 4 + kt) * 512 + 128 * m
                                nc.tensor.matmul(prz[:, 8 * m:8 * m + 8],
                                                 Wrz[:, base:base + 128],
                                                 rhs, start=(kt == 0),
                                                 stop=(kt == 3))
                        pin = psin.tile([128, 16], F32, tag="inn")
                        phn = pshn.tile([128, 16], F32, tag="hn")
                        for m in range(2):
                            for kt in range(2):
                                basei = (gi * 4 + kt) * 256 + 128 * m
                                baseh = (gi * 4 + 2 + kt) * 256 + 128 * m
                                nc.tensor.matmul(pin[:, 8 * m:8 * m + 8],
                                                 Wihn[:, basei:basei + 128],
                                                 xin[:, 8 * kt:8 * kt + 8],
                                                 start=(kt == 0), stop=(kt == 1))
                                nc.tensor.matmul(phn[:, 8 * m:8 * m + 8],
                                                 Wihn[:, baseh:baseh + 128],
                                                 h[gi][:, 8 * kt:8 * kt + 8],
                                                 start=(kt == 0), stop=(kt == 1))
                        rzs = actp.tile([128, 32], F32, tag="rzs")
                        nc.scalar.activation(rzs[:], prz[:], AF.Sigmoid)
                        tt = actp.tile([128, 16], F32, tag="tt")
                        nc.vector.tensor_tensor(out=tt[:], in0=rzs[:, 0:16],
                                                in1=phn[:], op=OP.mult)
                        pren = actp.tile([128, 16], F32, tag="pren")
                        nc.vector.tensor_tensor(out=pren[:], in0=pin[:],
                                                in1=tt[:], op=OP.add)
                        nt = actp.tile([128, 16], F32, tag="nt")
                        nc.scalar.activation(nt[:], pren[:], AF.Tanh)
                        hmn = actp.tile([128, 16], F32, tag="hmn")
                        nc.vector.tensor_tensor(out=hmn[:], in0=h[gi][:],
                                                in1=nt[:], op=OP.subtract)
                        zh = actp.tile([128, 16], F32, tag="zh")
                        nc.vector.tensor_tensor(out=zh[:], in0=rzs[:, 16:32],
                                                in1=hmn[:], op=OP.mult)
                        nc.vector.tensor_tensor(out=h[gi][:], in0=nt[:],
                                                in1=zh[:], op=OP.add)
                        pgl = glu_mm(h[gi], 3 + gi)
                        sgl = actp.tile([128, 16], F32, tag="sgl")
                        nc.scalar.activation(sgl[:], pgl[:], AF.Sigmoid)
                        nc.vector.tensor_tensor(out=gvec[gi][:], in0=h[gi][:],
                                                in1=sgl[:], op=OP.mult)
                        xin = gvec[gi]

                    # ---- output heads ----
                    po = pssm.tile([48, 128], F32, tag="sm")
                    cats = [gvec[0], gvec[1], gvec[2], d2t]
                    for kt in range(8):
                        src = cats[kt // 2][:, 8 * (kt % 2):8 * (kt % 2) + 8]
                        nc.tensor.matmul(po[0:40, 16:24],
                                         Wsig[:, 40 * kt:40 * (kt + 1)], src,
                                         start=(kt == 0), stop=(kt == 7))
                    for kt in range(8):
                        src = cats[kt // 2][:, 8 * (kt % 2):8 * (kt % 2) + 8]
                        nc.tensor.matmul(po[0:1, 24:32], Wgain[:, kt:kt + 1], src,
                                         start=(kt == 0), stop=(kt == 7))
                    sigt = actp.tile([40, 8], F32, tag="sigt")
                    nc.scalar.activation(sigt[:], po[0:40, 16:24], AF.Tanh)
                    pgw = actp.tile([1, 32], F32, tag="pgw")
                    nc.scalar.activation(pgw[:, 0:8], po[0:1, 24:32], AF.Sigmoid,
                                         bias=Bg[:, 0:1])
                    nc.vector.tensor_scalar(pgw[:, 8:16], pgw[:, 0:8], -1.0, 1.0,
                                            OP.mult, OP.add)
                    nc.vector.reciprocal(pgw[:, 16:24], pgw[:, 8:16])
                    nc.vector.tensor_tensor(out=pgw[:, 24:32], in0=pgw[:, 0:8],
                                            in1=pgw[:, 16:24], op=OP.mult)
                    ppg = pssm.tile([48, 128], F32, tag="sm")
                    nc.tensor.matmul(ppg[0:40, 32:40], ones40[:], pgw[:, 24:32],
                                     start=True, stop=True)
                    t4 = actp.tile([40, 8], F32, tag="t4")
                    nc.vector.tensor_tensor(out=t4[:], in0=ppg[0:40, 32:40],
                                            in1=slab[0:40, 16 * sa:16 * sa + 8],
                                            op=OP.mult)
                    t5 = actp.tile([40, 8], F32, tag="t5")
                    nc.vector.tensor_tensor(out=t5[:], in0=t4[:], in1=sigt[:],
                                            op=OP.add)
                    nc.vector.tensor_tensor(out=sigT[:], in0=t5[:],
                                            in1=gb_st[:, 16 * j:16 * j + 8],
                                            op=OP.mult)
                    pst = pssm.tile([48, 128], F32, tag="sm")
                    nc.tensor.transpose(out=pst[0:8, 64:104], in_=sigT[:],
                                        identity=ident[:40, :40])
                    sig_sb = actp.tile([8, 40], F32, tag="sigsb")
                    nc.scalar.copy(sig_sb[:], pst[0:8, 64:104])
                    if ci is None:
                        dst = buf[:, 256 + 40 * j:296 + 40 * j]
                    else:
                        dst = buf[:, ds(ci * (40 * chunk) + 40 * j + 256, 40)]
                    sd = nc.sync.dma_start(dst, sig_sb[:])
                    last_sig_dma[0] = sd
                    if debug and ci is None and j == 0:
                        for di, tt_ in enumerate([t1, fout, d2t, h[0], gvec[0],
                                                  gvec[2]]):
                            nc.sync.dma_start(dbg[di][:, 0:16], tt_[:, 0:16])
                        nc.sync.dma_start(dbg[6][0:40, 0:8], sigT[:])
                        nc.sync.dma_start(dbg[7][0:128, 0:16], slab[:, 0:16])

                def emit_chunk(ci):
                    if ci is None:
                        gt_src = gtab_d[0:chunk, :]
                        gi_src = gi8_d[:, 0:chunk]
                        off_src = offs_d[:, 0:chunk]
                    else:
                        gt_src = gtab_d[ds(ci * chunk, chunk), :]
                        gi_src = gi8_d[:, ds(ci * chunk, chunk)]
                        off_src = offs_d[:, ds(ci * chunk, chunk)]
                    nc.sync.dma_start(gb_st[:], bcast_part(gt_src, 40))
                    nc.sync.dma_start(gi_st[:], gi_src)
                    nc.sync.dma_start(offs_st[:], off_src)
                    for j in range(chunk):
                        emit_step(ci, j)

                nchunks = steps // chunk if run_chunks is None else run_chunks
                HINTS = (mybir.EngineType.PE, mybir.EngineType.Activation,
                         mybir.EngineType.DVE, mybir.EngineType.Pool,
                         mybir.EngineType.SP) if hints else ()
                kw = dict(hint_engines=HINTS, staggered_reset=sreset)
                if nchunks == 1:
                    emit_chunk(None)
                elif repeats == 1:
                    with tc.For_i(0, nchunks, 1, **kw) as ci:
                        emit_chunk(ci)
                else:
                    with tc.For_i(0, repeats, 1):
                        with tc.For_i(0, nchunks, 1, **kw) as ci:
                            emit_chunk(ci)

    nc.compile()
    orig = nc.to_json_bytes
    nc.to_json_bytes = lambda: _legalize_bir_json(orig())
    return nc


_NC_CACHE = {}


def kernel(**inputs):
    from concourse.bass_utils import run_bass_kernel_spmd

    key = (STEPS, CHUNK)
    if key not in _NC_CACHE:
        _NC_CACHE[key] = build_nc(STEPS, CHUNK)
    nc = _NC_CACHE[key]
    in_maps = host_prep(inputs, STEPS)
    res = run_bass_kernel_spmd(nc, in_maps, list(range(NCORE)))
    out = np.concatenate([res.results[c]["buf"][:, 256:] for c in range(NCORE)], 0)
    return out.astype(np.float32)
